# revision 7
# baseline (speedup 1.0000x reference)
"""Trainium2 Bass kernel: 2-layer mean-aggregation GraphSAGE encoder.

Problem (nn_BipartiteEncoder): N=50000 nodes, E=800000 edges,
    h   = relu(mean_agg(x)  @ W1_nbr.T + x @ W1_root.T + b1)
    out = mean_agg(h @ W2_nbr.T) + h @ W2_root.T + b2
(uses linearity: layer 2's neighbor transform is applied BEFORE
aggregation so both per-edge gathers move d=128 features).

Sharding: nodes are partitioned into 8 contiguous destination shards; each
core owns all edges whose dst lies in its shard.  Per-edge source features
are gathered from a full replica of the source table (x for layer 1, the
all-gathered q = h @ W2_nbr.T for layer 2) with `dma_gather`.  The
segment-sum is a one-hot matmul on the tensor engine: per 128-edge chunk a
[128 edge, 128 dst] 0/1 indicator (vector-engine is_equal against an iota
row, built 4 chunks per instruction) matmuls the gathered messages,
accumulating node-major [dst, feat] tiles in PSUM.  Mean normalisation is
a per-partition 1/deg scale folded into the PSUM->SBUF copy on the scalar
engine.

Performance structure (what makes this fast):
  * dma_gather descriptor generation runs on ONE Q7 cpu pair selected by
    `queue_num` (ucode: cpu_id/2 == queue_num); with num_swdge_queues=4 and
    calls rotated across queues 0-3, FOUR gathers generate descriptors
    concurrently (queue 0 synchronously on the engine, 1-3 async on their
    pairs), ~4x the single-queue ~8ns/edge rate that bounds the baseline.
  * both gather tables (x and the all-gathered q) are bf16, halving the
    per-descriptor DMA-engine work and the AllGather bytes; all matmul
    operands (messages, one-hot, weights, hT) are bf16 (PE fp32 is
    quarter-rate), accumulation stays fp32 in PSUM.
  * the inter-layer AllGather of q is SPLIT into 4 collectives over
    consecutive dst-tile ranges so all but the small last split overlap
    the layer-1 gather stream.

int16 gather indices require splitting each tile's edges into a lo
(src < 32768) and hi (rebased) gather; edges in the overlap zone
[hibase, 32768) are assigned to either side to balance the fixed per-tile
slot budgets that keep the single SPMD program valid on every core.
Layer 2 gathers by position in the split-AllGather output layout, so it
has its own index pack.  All index bookkeeping happens host-side on
edge_index only; all FLOPs on feature data run on the NeuronCores.
"""

import numpy as np
import ml_dtypes

import concourse.bass as bass
from concourse import bacc, mybir, tile
from concourse import bass_utils
from concourse.masks import make_identity

P = 128
F32 = mybir.dt.float32
BF16 = mybir.dt.bfloat16
I16 = mybir.dt.int16
BF = ml_dtypes.bfloat16
OVL = 10000          # lo/hi overlap zone width (rows below 32768)
KOH = 4              # one-hot chunks built per is_equal instruction
NQ = 4               # SWDGE queues (ucode MAX_SWDGE_QUEUES)


class Cfg:
    def __init__(self, n=50000, n_cores=8, in_dim=128, hid=256, out_dim=128,
                 split=32768, g_tiles=2):
        assert n % n_cores == 0
        assert in_dim == P and out_dim == P and hid == 2 * P
        self.n = n
        self.n_cores = n_cores
        self.in_dim, self.hid, self.out_dim = in_dim, hid, out_dim
        self.split = split
        self.hibase = split - OVL if n > split else n  # hi gather base row
        self.g = g_tiles
        self.npc = n // n_cores               # nodes per core
        self.nt = -(-self.npc // P)           # dst tiles per core
        self.npad = self.nt * P
        # AllGather splits: consecutive dst-tile ranges, small last split.
        # Many small splits let the serial CC chain start early (overlapping
        # the layer-1 gather stream) and leave only a tiny post-layer-1 tail.
        # split boundary at 32768 rows*n_cores: lo gathers (idx < 32768)
        # depend only on splits 0+1, so layer-2 lo-phase starts right as
        # layer 1 ends; hi-phase waits for the (early-finishing) tail splits.
        last = max(1, self.nt // 24)
        lo_t = min(self.split // (P * n_cores) * 1, self.nt)  # 32 tiles
        assert lo_t % 2 == 0
        r = self.nt - lo_t - last
        self.ag_tiles = [lo_t // 2, lo_t // 2, r, last]
        b = np.cumsum([0] + self.ag_tiles)
        self.ag_row = [(int(b[s]) * P, int(b[s + 1]) * P)
                       for s in range(len(self.ag_tiles))]
        self.newpos = None

    def split_of_tile(self, t):
        b = np.cumsum([0] + self.ag_tiles)
        for s in range(len(self.ag_tiles)):
            if t < b[s + 1]:
                return s
        raise ValueError(t)

    def pos_of(self, g):
        """position of padded node-slot g in the split-AllGather q layout"""
        r = g // self.npad
        l = g % self.npad
        pos = np.zeros_like(g)
        for s, (a, bb) in enumerate(self.ag_row):
            m = (l >= a) & (l < bb)
            rows = bb - a
            pos[m] = self.n_cores * a + r[m] * rows + (l[m] - a)
        return pos

    def key(self):
        return (self.n, self.n_cores, self.split, self.g)


def _layout(cfg, L, H):
    """Edge-slot layout per core: groups of up to `g` dst tiles, each group
    holding [lo slots (gg*L)] then [hi slots (gg*H)]."""
    groups = []
    off = 0
    for t0 in range(0, cfg.nt, cfg.g):
        gg = min(cfg.g, cfg.nt - t0)
        groups.append((t0, gg, off, off + gg * L))
        off += gg * (L + H)
    return groups, off


# --------------------------------------------------------------------------
# host-side preprocessing (edge_index only)
# --------------------------------------------------------------------------

def _pack_layer(cfg, src, core, tl, dit):
    """Bucket one layer's edges into the fixed slot grid.
    Returns (L, H, per-core list of (eidx [128, tot/16], dloc [128, nch]))."""
    n = cfg.n
    has_hi = n > cfg.split
    nk = cfg.n_cores * cfg.nt
    key = core * cfg.nt + tl
    cnt_all = np.bincount(key, minlength=nk)
    if has_hi:
        cnt_mlo = np.bincount(key[src < cfg.hibase], minlength=nk)
        cnt_mhi = np.bincount(key[src >= cfg.split], minlength=nk)
        T = max(2 * P, int(-(-cnt_all.max() // P) * P))
        L = max(P, int(-(-cnt_mlo.max() // P) * P))
        H = max(P, int(-(-cnt_mhi.max() // P) * P), T - L)
    else:
        L = max(P, int(-(-cnt_all.max() // P) * P))
        H = 0

    groups, tot = _layout(cfg, L, H)
    lo_base = np.zeros(cfg.nt, np.int64)
    hi_base = np.zeros(cfg.nt, np.int64)
    for (t0, gg, lo0, hi0) in groups:
        for ti in range(gg):
            lo_base[t0 + ti] = lo0 + ti * L
            hi_base[t0 + ti] = hi0 + ti * H

    packs = []
    for c in range(cfg.n_cores):
        m = core == c
        esrc, etl, edit = src[m], tl[m], dit[m]
        if has_hi:
            zone = np.where(esrc < cfg.hibase, 0,
                            np.where(esrc < cfg.split, 1, 2))
            seg = etl * 4 + zone
            order = np.argsort(seg, kind="stable")
            s_tl = etl[order]
            s_zone = zone[order]
            tstart = np.searchsorted(s_tl, np.arange(cfg.nt))
            pos_in_tile = np.arange(len(s_tl)) - tstart[s_tl]
            is_lo = (pos_in_tile < L) & (s_zone < 2)
            part_key = s_tl * 2 + (~is_lo).astype(np.int64)
            order2 = np.argsort(part_key, kind="stable")
            pk = part_key[order2]
            pstart = np.searchsorted(pk, np.arange(2 * cfg.nt))
            rank = np.arange(len(pk)) - pstart[pk]
            base = np.empty(2 * cfg.nt, np.int64)
            base[0::2] = lo_base
            base[1::2] = hi_base
            slots = base[pk] + rank
            fin = order[order2]
            lo_flag = is_lo[order2]
            vals = np.where(lo_flag, esrc[fin], esrc[fin] - cfg.hibase)
            assert rank[lo_flag].max(initial=0) < L
            assert rank[~lo_flag].max(initial=0) < max(H, 1)
        else:
            order = np.argsort(etl, kind="stable")
            s_tl = etl[order]
            tstart = np.searchsorted(s_tl, np.arange(cfg.nt))
            rank = np.arange(len(s_tl)) - tstart[s_tl]
            slots = lo_base[s_tl] + rank
            fin = order
            vals = esrc[fin]
            assert rank.max(initial=0) < L

        eidx = np.zeros(tot, np.int64)        # pad -> source row 0 (valid)
        dloc = np.full(tot, -1.0, np.float32)  # pad -> no one-hot match
        eidx[slots] = vals
        dloc[slots] = edit[fin]
        assert 0 <= eidx.min() and eidx.max() < cfg.split

        e16 = np.tile(eidx.astype(np.int16).reshape(-1, 16).T, (8, 1))
        packs.append((np.ascontiguousarray(e16),
                      np.ascontiguousarray(dloc.reshape(-1, P).T.astype(BF))))
    return L, H, packs


def pack_inputs(cfg, x, edge_index, w1n, w1r, b1, w2n, w2r, b2):
    x = np.ascontiguousarray(np.asarray(x, np.float32))
    src = np.asarray(edge_index[0], np.int64)
    dst = np.asarray(edge_index[1], np.int64)
    n = cfg.n

    deg = np.bincount(dst, minlength=n).astype(np.float32)
    deginv = np.where(deg > 0,
                      np.float32(1.0) / np.maximum(deg, np.float32(1.0)),
                      np.float32(0.0)).astype(np.float32)

    # degree-balanced assignment of dst nodes to the 128-slot (core,tile)
    # bins: equalises per-tile edge counts, which sets the SPMD slot budget.
    nbins = cfg.n_cores * cfg.nt
    order = np.argsort(-deg, kind="stable")
    r = np.arange(n)
    p_pass = r // nbins
    j = r % nbins
    b = np.where(p_pass % 2 == 0, j, nbins - 1 - j)
    newpos = np.empty(n, np.int64)
    newpos[order] = b * P + p_pass
    cfg.newpos = newpos

    npd = newpos[dst]
    core = npd // cfg.npad
    dlocal = npd % cfg.npad
    tl = dlocal // P
    dit = (dlocal % P).astype(np.float32)

    L1, H1, packs1 = _pack_layer(cfg, src, core, tl, dit)
    L2, H2, packs2 = _pack_layer(cfg, cfg.pos_of(newpos[src]), core, tl, dit)

    w1n = np.asarray(w1n, np.float32)
    w1r = np.asarray(w1r, np.float32)
    w2n = np.asarray(w2n, np.float32)
    w2r = np.asarray(w2r, np.float32)
    b1 = np.asarray(b1, np.float32)
    b2 = np.asarray(b2, np.float32)
    shared = {
        "x_full": x.astype(BF),
        "w1nT": np.ascontiguousarray(w1n.T).astype(BF),
        "w1rT": np.ascontiguousarray(w1r.T).astype(BF),
        "b1c": np.ascontiguousarray(b1.reshape(2, P).T),
        "w2nT": np.ascontiguousarray(
            np.concatenate([w2n.T[:P, :], w2n.T[P:, :]], axis=1)).astype(BF),
        "w2rT": np.ascontiguousarray(
            np.concatenate([w2r.T[:P, :], w2r.T[P:, :]], axis=1)).astype(BF),
        "b2r": np.ascontiguousarray(b2.reshape(1, P)).astype(BF),
        "iota": np.tile(np.arange(P, dtype=np.float32), (P, 1)).astype(BF),
    }

    slotnode = np.full(cfg.n_cores * cfg.npad, -1, np.int64)
    slotnode[newpos] = np.arange(n)

    # head-start split of the layer-1 index array: group 0 loads first
    g0_slots = min(cfg.g, cfg.nt) * (L1 + H1)

    in_maps = []
    for c in range(cfg.n_cores):
        e1, d1 = packs1[c]
        e2, d2 = packs2[c]
        sl = slotnode[c * cfg.npad:(c + 1) * cfg.npad]
        m_sl = sl >= 0
        xt = np.zeros((P, cfg.npad), np.float32)
        xt[:, m_sl] = x[sl[m_sl]].T
        dinv = np.zeros((P, cfg.nt), np.float32)
        dinv.T.flat[m_sl] = deginv[sl[m_sl]]

        im = dict(shared)
        im["eidx1a"] = np.ascontiguousarray(e1[:, :g0_slots // 16])
        im["eidx1b"] = np.ascontiguousarray(e1[:, g0_slots // 16:])
        im["dloc1"] = d1
        im["eidx2"] = e2
        im["dloc2"] = d2
        im["dinv"] = dinv
        im["xtile"] = xt.astype(BF)
        in_maps.append(im)

    return (L1, H1, L2, H2), in_maps


# --------------------------------------------------------------------------
# Bass program
# --------------------------------------------------------------------------

def build_program(cfg, budgets):
    L1, H1, L2, H2 = budgets
    nc = bacc.Bacc("TRN2", target_bir_lowering=False, debug=False,
                   enable_asserts=False, num_devices=cfg.n_cores,
                   num_swdge_queues=NQ)
    groups1, tot1 = _layout(cfg, L1, H1)
    groups2, tot2 = _layout(cfg, L2, H2)
    npc, nt, npad = cfg.npc, cfg.nt, cfg.npad
    g0_slots = min(cfg.g, cfg.nt) * (L1 + H1)

    x_full = nc.dram_tensor("x_full", [cfg.n, P], BF16, kind="ExternalInput")
    xtile_d = nc.dram_tensor("xtile", [P, npad], BF16, kind="ExternalInput")
    w1n_d = nc.dram_tensor("w1nT", [P, cfg.hid], BF16, kind="ExternalInput")
    w1r_d = nc.dram_tensor("w1rT", [P, cfg.hid], BF16, kind="ExternalInput")
    b1_d = nc.dram_tensor("b1c", [P, 2], F32, kind="ExternalInput")
    w2n_d = nc.dram_tensor("w2nT", [P, 2 * P], BF16, kind="ExternalInput")
    w2r_d = nc.dram_tensor("w2rT", [P, 2 * P], BF16, kind="ExternalInput")
    b2_d = nc.dram_tensor("b2r", [1, P], BF16, kind="ExternalInput")
    iota_d = nc.dram_tensor("iota", [P, P], BF16, kind="ExternalInput")
    e1a_d = nc.dram_tensor("eidx1a", [P, g0_slots // 16], I16, kind="ExternalInput")
    e1b_d = nc.dram_tensor("eidx1b", [P, (tot1 - g0_slots) // 16], I16,
                           kind="ExternalInput")
    d1_d = nc.dram_tensor("dloc1", [P, tot1 // P], BF16, kind="ExternalInput")
    e2_d = nc.dram_tensor("eidx2", [P, tot2 // 16], I16, kind="ExternalInput")
    d2_d = nc.dram_tensor("dloc2", [P, tot2 // P], BF16, kind="ExternalInput")
    dinv_d = nc.dram_tensor("dinv", [P, nt], F32, kind="ExternalInput")
    out_d = nc.dram_tensor("out", [npad, P], F32, kind="ExternalOutput")

    AF = mybir.ActivationFunctionType
    OP = mybir.AluOpType
    qctr = [0]

    def next_q():
        q = qctr[0] % NQ
        qctr[0] += 1
        return q

    with tile.TileContext(nc) as tc:
        with (tc.tile_pool(name="const", bufs=1) as cp,
              tc.tile_pool(name="dram", bufs=1, space="DRAM") as dp,
              tc.tile_pool(name="msg", bufs=4) as mp,
              tc.tile_pool(name="oh", bufs=4) as ohp,
              tc.tile_pool(name="stage", bufs=3) as sp):
            eidx1a = cp.tile([P, g0_slots // 16], I16)
            nc.sync.dma_start(out=eidx1a[:], in_=e1a_d.ap())
            dloc1 = cp.tile([P, tot1 // P], BF16)
            nc.sync.dma_start(out=dloc1[:], in_=d1_d.ap())
            iota = cp.tile([P, P], BF16)
            nc.sync.dma_start(out=iota[:], in_=iota_d.ap())
            eidx1b = cp.tile([P, (tot1 - g0_slots) // 16], I16)
            nc.sync.dma_start(out=eidx1b[:], in_=e1b_d.ap())
            xt = cp.tile([P, npad], BF16)
            nc.sync.dma_start(out=xt[:], in_=xtile_d.ap())
            w1n = cp.tile([P, cfg.hid], BF16)
            nc.sync.dma_start(out=w1n[:], in_=w1n_d.ap())
            w1r = cp.tile([P, cfg.hid], BF16)
            nc.sync.dma_start(out=w1r[:], in_=w1r_d.ap())
            b1c = cp.tile([P, 2], F32)
            nc.sync.dma_start(out=b1c[:], in_=b1_d.ap())
            w2n = cp.tile([P, 2 * P], BF16)
            nc.sync.dma_start(out=w2n[:], in_=w2n_d.ap())
            w2r = cp.tile([P, 2 * P], BF16)
            nc.sync.dma_start(out=w2r[:], in_=w2r_d.ap())
            b2r = cp.tile([1, P], BF16)
            nc.sync.dma_start(out=b2r[:], in_=b2_d.ap())
            eidx2 = cp.tile([P, tot2 // 16], I16)
            nc.sync.dma_start(out=eidx2[:], in_=e2_d.ap())
            dloc2 = cp.tile([P, tot2 // P], BF16)
            nc.sync.dma_start(out=dloc2[:], in_=d2_d.ap())
            dinv = cp.tile([P, nt], F32)
            nc.sync.dma_start(out=dinv[:], in_=dinv_d.ap())
            ones1 = cp.tile([1, P], BF16)
            nc.vector.memset(ones1[:], 1.0)
            ident = cp.tile([P, P], BF16)
            make_identity(nc, ident[:])
            hT = cp.tile([P, 2 * npad], BF16)
            plo = cp.tile([P, npad], F32)

            q_locs = [dp.tile([b - a, P], BF16, name=f"qloc{s}")
                      for s, (a, b) in enumerate(cfg.ag_row)]
            q_all = dp.tile([cfg.n_cores * cfg.npad, P], BF16,
                            addr_space="Local")

            def gather_group(layer, g, src_lo, src_hi):
                groups, L, H = (groups1, L1, H1) if layer == 1 else (groups2, L2, H2)
                t0, gg, lo0, hi0 = groups[g]
                if layer == 1:
                    if g == 0:
                        elo = eidx1a[:, lo0 // 16:(lo0 + gg * L) // 16]
                        ehi = eidx1a[:, hi0 // 16:(hi0 + gg * H) // 16] if H else None
                    else:
                        elo = eidx1b[:, (lo0 - g0_slots) // 16:
                                     (lo0 - g0_slots + gg * L) // 16]
                        ehi = (eidx1b[:, (hi0 - g0_slots) // 16:
                                      (hi0 - g0_slots + gg * H) // 16] if H else None)
                else:
                    elo = eidx2[:, lo0 // 16:(lo0 + gg * L) // 16]
                    ehi = eidx2[:, hi0 // 16:(hi0 + gg * H) // 16] if H else None
                mlo = mhi = None
                if src_lo is not None:
                    mlo = mp.tile([P, gg * max(L1, L2)], BF16, tag="mlo")
                    nc.gpsimd.dma_gather(
                        mlo[:, :gg * L].rearrange("p (c e) -> p c e", e=P),
                        src_lo, elo, gg * L, gg * L, P, single_packet=False,
                        queue_num=next_q())
                if H and src_hi is not None:
                    mhi = mp.tile([P, gg * max(H1, H2)], BF16, tag="mhi")
                    nc.gpsimd.dma_gather(
                        mhi[:, :gg * H].rearrange("p (c e) -> p c e", e=P),
                        src_hi, ehi, gg * H, gg * H, P, single_packet=False,
                        queue_num=next_q())
                return mlo, mhi

            def onehot_slab(dloc_t, gc, k):
                """one is_equal builds one-hots for chunks gc..gc+k-1"""
                oh = ohp.tile([P, KOH * P], BF16, tag="oh")
                nc.vector.tensor_tensor(
                    out=oh[:, :k * P].rearrange("p (c e) -> p c e", e=P),
                    in0=iota[:].rearrange("p (o e) -> p o e", o=1)
                        .to_broadcast([P, k, P]),
                    in1=dloc_t[:, gc:gc + k].to_broadcast([P, k, P]),
                    op=OP.is_equal)
                return oh

            def aggregate(layer, g, ti, mlo, mhi, pp, tag, which="both"):
                groups, L, H = (groups1, L1, H1) if layer == 1 else (groups2, L2, H2)
                dloc_t = dloc1 if layer == 1 else dloc2
                t0, gg, lo0, hi0 = groups[g]
                lch, hch = L // P, H // P
                # (msg tile, local chunk, global dloc chunk) runs
                runs = []
                if which in ("both", "lo"):
                    runs.append((mlo, ti * lch, lo0 // P + ti * lch, lch))
                if H and which in ("both", "hi"):
                    runs.append((mhi, ti * hch, hi0 // P + ti * hch, hch))
                ps = pp.tile([P, P], F32, tag=tag, name=f"{tag}{g}_{ti}")
                nch = sum(r[3] for r in runs)
                k = 0
                for (mt, lc0, gc0, cnt) in runs:
                    for j0 in range(0, cnt, KOH):
                        kk = min(KOH, cnt - j0)
                        oh = onehot_slab(dloc_t, gc0 + j0, kk)
                        for j in range(kk):
                            lc = lc0 + j0 + j
                            nc.tensor.matmul(ps[:], lhsT=oh[:, j * P:(j + 1) * P],
                                             rhs=mt[:, lc * P:(lc + 1) * P],
                                             start=(k == 0), stop=(k == nch - 1))
                            k += 1
                return ps

            def layer1_tile(g, ti, mlo, mhi, pp):
                t = groups1[g][0] + ti
                rows = P
                ps_a = aggregate(1, g, ti, mlo, mhi, pp, "psa")
                agg_nm = sp.tile([P, P], BF16, tag="aggnm")
                nc.scalar.activation(agg_nm[:], ps_a[:], AF.Copy,
                                     scale=dinv[:, t:t + 1])
                ps_t = pp.tile([P, P], BF16, tag="pst", name=f"pst{t}")
                nc.tensor.transpose(ps_t[:], agg_nm[:], ident[:])
                aggrT = sp.tile([P, P], BF16, tag="aggrT")
                nc.scalar.activation(aggrT[:], ps_t[:], AF.Copy)
                for h in range(2):
                    ps_h = pp.tile([P, P], F32, tag="psh", name=f"psh{t}_{h}")
                    nc.tensor.matmul(ps_h[:], lhsT=w1n[:, h * P:(h + 1) * P],
                                     rhs=aggrT[:], start=True, stop=False)
                    nc.tensor.matmul(ps_h[:], lhsT=w1r[:, h * P:(h + 1) * P],
                                     rhs=xt[:, t * P:(t + 1) * P],
                                     start=False, stop=True)
                    nc.scalar.activation(hT[:, h * npad + t * P:h * npad + (t + 1) * P],
                                         ps_h[:], AF.Relu, bias=b1c[:, h:h + 1])
                ps_q = pp.tile([P, P], F32, tag="psq", name=f"psq{t}")
                nc.tensor.matmul(ps_q[:], lhsT=hT[:, t * P:(t + 1) * P],
                                 rhs=w2n[:, 0:P], start=True, stop=False)
                nc.tensor.matmul(ps_q[:], lhsT=hT[:, npad + t * P:npad + (t + 1) * P],
                                 rhs=w2n[:, P:2 * P], start=False, stop=True)
                qsb = sp.tile([P, P], BF16, tag="qsb")
                nc.scalar.activation(qsb[:], ps_q[:], AF.Copy)
                s = cfg.split_of_tile(t)
                a, b = cfg.ag_row[s]
                r0 = t * P - a
                nc.sync.dma_start(out=q_locs[s][r0:r0 + rows, :],
                                  in_=qsb[:rows, :])

            def layer2_lo_tile(g, ti, mlo, pp):
                # lo-source aggregate + root term + bias -> plo stash
                t = groups2[g][0] + ti
                ps_g = aggregate(2, g, ti, mlo, None, pp, "psg", which="lo")
                agg_sb = sp.tile([P, P], F32, tag="aggsb")
                nc.scalar.activation(agg_sb[:], ps_g[:], AF.Copy,
                                     scale=dinv[:, t:t + 1])
                ps_r = pp.tile([P, P], F32, tag="psr", name=f"psr{t}")
                nc.tensor.matmul(ps_r[:], lhsT=hT[:, t * P:(t + 1) * P],
                                 rhs=w2r[:, 0:P], start=True, stop=False)
                nc.tensor.matmul(ps_r[:], lhsT=hT[:, npad + t * P:npad + (t + 1) * P],
                                 rhs=w2r[:, P:2 * P], start=False, stop=False)
                nc.tensor.matmul(ps_r[:], lhsT=ones1[:], rhs=b2r[:],
                                 start=False, stop=True)
                nc.vector.tensor_tensor(out=plo[:, t * P:(t + 1) * P],
                                        in0=agg_sb[:], in1=ps_r[:], op=OP.add)

            def layer2_hi_tile(g, ti, mhi, pp):
                t = groups2[g][0] + ti
                rows = P
                ps_g = aggregate(2, g, ti, None, mhi, pp, "psg", which="hi")
                agg_sb = sp.tile([P, P], F32, tag="aggsb")
                nc.scalar.activation(agg_sb[:], ps_g[:], AF.Copy,
                                     scale=dinv[:, t:t + 1])
                osb = sp.tile([P, P], F32, tag="osb")
                nc.vector.tensor_tensor(out=osb[:], in0=agg_sb[:],
                                        in1=plo[:, t * P:(t + 1) * P],
                                        op=OP.add)
                nc.sync.dma_start(out=out_d.ap()[t * P:t * P + rows, :],
                                  in_=osb[:rows, :])

            with tc.tile_pool(name="ps1", bufs=2, space="PSUM") as pp1:
                for g in range(len(groups1)):
                    mlo, mhi = gather_group(
                        1, g, x_full.ap(),
                        x_full.ap()[cfg.hibase:, :] if H1 else None)
                    for ti in range(groups1[g][1]):
                        layer1_tile(g, ti, mlo, mhi, pp1)

            for s in range(len(cfg.ag_row)):
                a, b = cfg.ag_row[s]
                nc.gpsimd.collective_compute(
                    "AllGather", mybir.AluOpType.bypass,
                    replica_groups=[list(range(cfg.n_cores))],
                    ins=[q_locs[s].opt()],
                    outs=[q_all[cfg.n_cores * a:cfg.n_cores * b, :].opt()])

            with tc.tile_pool(name="ps2", bufs=3, space="PSUM") as pp2:
                for g in range(len(groups2)):
                    mlo, _ = gather_group(2, g, q_all[:cfg.split, :], None)
                    for ti in range(groups2[g][1]):
                        layer2_lo_tile(g, ti, mlo, pp2)
                for g in range(len(groups2)):
                    _, mhi = gather_group(2, g, None, q_all[cfg.hibase:, :])
                    for ti in range(groups2[g][1]):
                        layer2_hi_tile(g, ti, mhi, pp2)

    nc.compile()
    return nc


# --------------------------------------------------------------------------
# entry point
# --------------------------------------------------------------------------

_CACHE = {}


def prepare(inputs, cfg=None):
    x = np.asarray(inputs["x"], np.float32)
    if cfg is None:
        cfg = Cfg(n=x.shape[0])
    budgets, in_maps = pack_inputs(
        cfg, x, inputs["edge_index"],
        inputs["W1_nbr"], inputs["W1_root"], inputs["b1"],
        inputs["W2_nbr"], inputs["W2_root"], inputs["b2"])
    key = (cfg.key(), budgets)
    nc = _CACHE.get(key)
    if nc is None:
        nc = build_program(cfg, budgets)
        _CACHE[key] = nc
    return nc, in_maps, cfg


def kernel(**inputs) -> np.ndarray:
    nc, in_maps, cfg = prepare(inputs)
    res = bass_utils.run_bass_kernel_spmd(
        nc, in_maps, core_ids=list(range(cfg.n_cores)))
    out = np.concatenate([res.results[c]["out"] for c in range(cfg.n_cores)],
                         axis=0)
    return np.ascontiguousarray(out[cfg.newpos], dtype=np.float32)


# revision 8
# speedup vs baseline: 1.1821x; 1.1821x over previous
"""Trainium2 Bass kernel: 2-layer mean-aggregation GraphSAGE encoder.

Problem (nn_BipartiteEncoder): N=50000 nodes, E=800000 edges,
    h   = relu(mean_agg(x)  @ W1_nbr.T + x @ W1_root.T + b1)
    out = mean_agg(h @ W2_nbr.T) + h @ W2_root.T + b2
(uses linearity: layer 2's neighbor transform is applied BEFORE
aggregation so both per-edge gathers move d=128 features).

Sharding: nodes are partitioned into 8 contiguous destination shards; each
core owns all edges whose dst lies in its shard.  Per-edge source features
are gathered from a full replica of the source table (x for layer 1, the
all-gathered q = h @ W2_nbr.T for layer 2) with `dma_gather`.  The
segment-sum is a one-hot matmul on the tensor engine: per 128-edge chunk a
[128 edge, 128 dst] 0/1 indicator (vector-engine is_equal against an iota
row, built 4 chunks per instruction) matmuls the gathered messages,
accumulating node-major [dst, feat] tiles in PSUM.  Mean normalisation is
a per-partition 1/deg scale folded into the PSUM->SBUF copy on the scalar
engine.

Performance structure (what makes this fast):
  * dma_gather descriptor generation runs on ONE Q7 cpu pair selected by
    `queue_num` (ucode: cpu_id/2 == queue_num); with num_swdge_queues=4 and
    calls rotated across queues 0-3, FOUR gathers generate descriptors
    concurrently (queue 0 synchronously on the engine, 1-3 async on their
    pairs), ~4x the single-queue ~8ns/edge rate that bounds the baseline.
  * both gather tables (x and the all-gathered q) are bf16, halving the
    per-descriptor DMA-engine work and the AllGather bytes; all matmul
    operands (messages, one-hot, weights, hT) are bf16 (PE fp32 is
    quarter-rate), accumulation stays fp32 in PSUM.
  * the inter-layer AllGather of q is SPLIT into 4 collectives over
    consecutive dst-tile ranges so all but the small last split overlap
    the layer-1 gather stream.

int16 gather indices require splitting each tile's edges into a lo
(src < 32768) and hi (rebased) gather; edges in the overlap zone
[hibase, 32768) are assigned to either side to balance the fixed per-tile
slot budgets that keep the single SPMD program valid on every core.
Layer 2 gathers by position in the split-AllGather output layout, so it
has its own index pack.  All index bookkeeping happens host-side on
edge_index only; all FLOPs on feature data run on the NeuronCores.
"""

import numpy as np
import ml_dtypes

import concourse.bass as bass
from concourse import bacc, mybir, tile
from concourse import bass_utils
from concourse.masks import make_identity

P = 128
F32 = mybir.dt.float32
BF16 = mybir.dt.bfloat16
I16 = mybir.dt.int16
BF = ml_dtypes.bfloat16
OVL = 10000          # lo/hi overlap zone width (rows below 32768)
KOH = 4              # one-hot chunks built per is_equal instruction
NQ = 4               # SWDGE queues (ucode MAX_SWDGE_QUEUES)


class Cfg:
    def __init__(self, n=50000, n_cores=8, in_dim=128, hid=256, out_dim=128,
                 split=32768, g_tiles=2):
        assert n % n_cores == 0
        assert in_dim == P and out_dim == P and hid == 2 * P
        self.n = n
        self.n_cores = n_cores
        self.in_dim, self.hid, self.out_dim = in_dim, hid, out_dim
        self.split = split
        self.hibase = split - OVL if n > split else n  # hi gather base row
        self.g = g_tiles
        self.npc = n // n_cores               # nodes per core
        self.nt = -(-self.npc // P)           # dst tiles per core
        self.npad = self.nt * P
        # AllGather splits: consecutive dst-tile ranges, small last split.
        # Many small splits let the serial CC chain start early (overlapping
        # the layer-1 gather stream) and leave only a tiny post-layer-1 tail.
        # split boundary at 32768 rows*n_cores: lo gathers (idx < 32768)
        # depend only on splits 0+1, so layer-2 lo-phase starts right as
        # layer 1 ends; hi-phase waits for the (early-finishing) tail splits.
        last = max(1, self.nt // 24)
        lo_t = min(self.split // (P * n_cores) * 1, self.nt)  # 32 tiles
        assert lo_t % 2 == 0
        r = self.nt - lo_t - last
        self.ag_tiles = [lo_t // 2, lo_t // 2, r, last]
        b = np.cumsum([0] + self.ag_tiles)
        self.ag_row = [(int(b[s]) * P, int(b[s + 1]) * P)
                       for s in range(len(self.ag_tiles))]
        self.newpos = None

    def split_of_tile(self, t):
        b = np.cumsum([0] + self.ag_tiles)
        for s in range(len(self.ag_tiles)):
            if t < b[s + 1]:
                return s
        raise ValueError(t)

    def pos_of(self, g):
        """position of padded node-slot g in the split-AllGather q layout"""
        r = g // self.npad
        l = g % self.npad
        pos = np.zeros_like(g)
        for s, (a, bb) in enumerate(self.ag_row):
            m = (l >= a) & (l < bb)
            rows = bb - a
            pos[m] = self.n_cores * a + r[m] * rows + (l[m] - a)
        return pos

    def key(self):
        return (self.n, self.n_cores, self.split, self.g)


def _layout(cfg, L, H):
    """Edge-slot layout per core: groups of up to `g` dst tiles, each group
    holding [lo slots (gg*L)] then [hi slots (gg*H)]."""
    groups = []
    off = 0
    for t0 in range(0, cfg.nt, cfg.g):
        gg = min(cfg.g, cfg.nt - t0)
        groups.append((t0, gg, off, off + gg * L))
        off += gg * (L + H)
    return groups, off


# --------------------------------------------------------------------------
# host-side preprocessing (edge_index only)
# --------------------------------------------------------------------------

def _pack_layer(cfg, src, core, tl, dit):
    """Bucket one layer's edges into the fixed slot grid.
    Returns (L, H, per-core list of (eidx [128, tot/16], dloc [128, nch]))."""
    n = cfg.n
    has_hi = n > cfg.split
    nk = cfg.n_cores * cfg.nt
    key = core * cfg.nt + tl
    cnt_all = np.bincount(key, minlength=nk)
    if has_hi:
        cnt_mlo = np.bincount(key[src < cfg.hibase], minlength=nk)
        cnt_mhi = np.bincount(key[src >= cfg.split], minlength=nk)
        T = max(2 * P, int(-(-cnt_all.max() // P) * P))
        L = max(P, int(-(-cnt_mlo.max() // P) * P))
        H = max(P, int(-(-cnt_mhi.max() // P) * P), T - L)
    else:
        L = max(P, int(-(-cnt_all.max() // P) * P))
        H = 0

    groups, tot = _layout(cfg, L, H)
    lo_base = np.zeros(cfg.nt, np.int64)
    hi_base = np.zeros(cfg.nt, np.int64)
    for (t0, gg, lo0, hi0) in groups:
        for ti in range(gg):
            lo_base[t0 + ti] = lo0 + ti * L
            hi_base[t0 + ti] = hi0 + ti * H

    packs = []
    for c in range(cfg.n_cores):
        m = core == c
        esrc, etl, edit = src[m], tl[m], dit[m]
        if has_hi:
            zone = np.where(esrc < cfg.hibase, 0,
                            np.where(esrc < cfg.split, 1, 2))
            seg = etl * 4 + zone
            order = np.argsort(seg, kind="stable")
            s_tl = etl[order]
            s_zone = zone[order]
            tstart = np.searchsorted(s_tl, np.arange(cfg.nt))
            pos_in_tile = np.arange(len(s_tl)) - tstart[s_tl]
            is_lo = (pos_in_tile < L) & (s_zone < 2)
            part_key = s_tl * 2 + (~is_lo).astype(np.int64)
            order2 = np.argsort(part_key, kind="stable")
            pk = part_key[order2]
            pstart = np.searchsorted(pk, np.arange(2 * cfg.nt))
            rank = np.arange(len(pk)) - pstart[pk]
            base = np.empty(2 * cfg.nt, np.int64)
            base[0::2] = lo_base
            base[1::2] = hi_base
            slots = base[pk] + rank
            fin = order[order2]
            lo_flag = is_lo[order2]
            vals = np.where(lo_flag, esrc[fin], esrc[fin] - cfg.hibase)
            assert rank[lo_flag].max(initial=0) < L
            assert rank[~lo_flag].max(initial=0) < max(H, 1)
        else:
            order = np.argsort(etl, kind="stable")
            s_tl = etl[order]
            tstart = np.searchsorted(s_tl, np.arange(cfg.nt))
            rank = np.arange(len(s_tl)) - tstart[s_tl]
            slots = lo_base[s_tl] + rank
            fin = order
            vals = esrc[fin]
            assert rank.max(initial=0) < L

        eidx = np.zeros(tot, np.int64)        # pad -> source row 0 (valid)
        dloc = np.full(tot, -1.0, np.float32)  # pad -> no one-hot match
        eidx[slots] = vals
        dloc[slots] = edit[fin]
        assert 0 <= eidx.min() and eidx.max() < cfg.split

        e16 = np.tile(eidx.astype(np.int16).reshape(-1, 16).T, (8, 1))
        packs.append((np.ascontiguousarray(e16),
                      np.ascontiguousarray(dloc.reshape(-1, P).T.astype(BF))))
    return L, H, packs


def pack_inputs(cfg, x, edge_index, w1n, w1r, b1, w2n, w2r, b2):
    x = np.ascontiguousarray(np.asarray(x, np.float32))
    src = np.asarray(edge_index[0], np.int64)
    dst = np.asarray(edge_index[1], np.int64)
    n = cfg.n

    deg = np.bincount(dst, minlength=n).astype(np.float32)
    deginv = np.where(deg > 0,
                      np.float32(1.0) / np.maximum(deg, np.float32(1.0)),
                      np.float32(0.0)).astype(np.float32)

    # degree-balanced assignment of dst nodes to the 128-slot (core,tile)
    # bins: equalises per-tile edge counts, which sets the SPMD slot budget.
    nbins = cfg.n_cores * cfg.nt
    order = np.argsort(-deg, kind="stable")
    r = np.arange(n)
    p_pass = r // nbins
    j = r % nbins
    b = np.where(p_pass % 2 == 0, j, nbins - 1 - j)
    newpos = np.empty(n, np.int64)
    newpos[order] = b * P + p_pass
    cfg.newpos = newpos

    npd = newpos[dst]
    core = npd // cfg.npad
    dlocal = npd % cfg.npad
    tl = dlocal // P
    dit = (dlocal % P).astype(np.float32)

    L1, H1, packs1 = _pack_layer(cfg, src, core, tl, dit)
    L2, H2, packs2 = _pack_layer(cfg, cfg.pos_of(newpos[src]), core, tl, dit)

    w1n = np.asarray(w1n, np.float32)
    w1r = np.asarray(w1r, np.float32)
    w2n = np.asarray(w2n, np.float32)
    w2r = np.asarray(w2r, np.float32)
    b1 = np.asarray(b1, np.float32)
    b2 = np.asarray(b2, np.float32)
    shared = {
        "x_full": x.astype(BF),
        "w1nT": np.ascontiguousarray(w1n.T).astype(BF),
        "w1rT": np.ascontiguousarray(w1r.T).astype(BF),
        "b1c": np.ascontiguousarray(b1.reshape(2, P).T),
        "w2nT": np.ascontiguousarray(
            np.concatenate([w2n.T[:P, :], w2n.T[P:, :]], axis=1)).astype(BF),
        "w2rT": np.ascontiguousarray(
            np.concatenate([w2r.T[:P, :], w2r.T[P:, :]], axis=1)).astype(BF),
        "b2r": np.ascontiguousarray(b2.reshape(1, P)).astype(BF),
        "iota": np.tile(np.arange(P, dtype=np.float32), (P, 1)).astype(BF),
    }

    slotnode = np.full(cfg.n_cores * cfg.npad, -1, np.int64)
    slotnode[newpos] = np.arange(n)

    # head-start split of the layer-1 index array: group 0 loads first
    g0_slots = min(cfg.g, cfg.nt) * (L1 + H1)

    in_maps = []
    for c in range(cfg.n_cores):
        e1, d1 = packs1[c]
        e2, d2 = packs2[c]
        sl = slotnode[c * cfg.npad:(c + 1) * cfg.npad]
        m_sl = sl >= 0
        xt = np.zeros((P, cfg.npad), np.float32)
        xt[:, m_sl] = x[sl[m_sl]].T
        dinv = np.zeros((P, cfg.nt), np.float32)
        dinv.T.flat[m_sl] = deginv[sl[m_sl]]

        im = dict(shared)
        im["eidx1a"] = np.ascontiguousarray(e1[:, :g0_slots // 16])
        im["eidx1b"] = np.ascontiguousarray(e1[:, g0_slots // 16:])
        im["dloc1"] = d1
        im["eidx2"] = e2
        im["dloc2"] = d2
        im["dinv"] = dinv
        im["xtile"] = xt.astype(BF)
        in_maps.append(im)

    return (L1, H1, L2, H2), in_maps


# --------------------------------------------------------------------------
# Bass program
# --------------------------------------------------------------------------

def build_program(cfg, budgets):
    L1, H1, L2, H2 = budgets
    nc = bacc.Bacc("TRN2", target_bir_lowering=False, debug=False,
                   enable_asserts=False, num_devices=cfg.n_cores,
                   num_swdge_queues=NQ)
    groups1, tot1 = _layout(cfg, L1, H1)
    groups2, tot2 = _layout(cfg, L2, H2)
    npc, nt, npad = cfg.npc, cfg.nt, cfg.npad
    g0_slots = min(cfg.g, cfg.nt) * (L1 + H1)

    x_full = nc.dram_tensor("x_full", [cfg.n, P], BF16, kind="ExternalInput")
    xtile_d = nc.dram_tensor("xtile", [P, npad], BF16, kind="ExternalInput")
    w1n_d = nc.dram_tensor("w1nT", [P, cfg.hid], BF16, kind="ExternalInput")
    w1r_d = nc.dram_tensor("w1rT", [P, cfg.hid], BF16, kind="ExternalInput")
    b1_d = nc.dram_tensor("b1c", [P, 2], F32, kind="ExternalInput")
    w2n_d = nc.dram_tensor("w2nT", [P, 2 * P], BF16, kind="ExternalInput")
    w2r_d = nc.dram_tensor("w2rT", [P, 2 * P], BF16, kind="ExternalInput")
    b2_d = nc.dram_tensor("b2r", [1, P], BF16, kind="ExternalInput")
    iota_d = nc.dram_tensor("iota", [P, P], BF16, kind="ExternalInput")
    e1a_d = nc.dram_tensor("eidx1a", [P, g0_slots // 16], I16, kind="ExternalInput")
    e1b_d = nc.dram_tensor("eidx1b", [P, (tot1 - g0_slots) // 16], I16,
                           kind="ExternalInput")
    d1_d = nc.dram_tensor("dloc1", [P, tot1 // P], BF16, kind="ExternalInput")
    e2_d = nc.dram_tensor("eidx2", [P, tot2 // 16], I16, kind="ExternalInput")
    d2_d = nc.dram_tensor("dloc2", [P, tot2 // P], BF16, kind="ExternalInput")
    dinv_d = nc.dram_tensor("dinv", [P, nt], F32, kind="ExternalInput")
    out_d = nc.dram_tensor("out", [npad, P], F32, kind="ExternalOutput")

    AF = mybir.ActivationFunctionType
    OP = mybir.AluOpType
    qctr = [0]

    def next_q():
        q = qctr[0] % NQ
        qctr[0] += 1
        return q

    with tile.TileContext(nc) as tc:
        with (tc.tile_pool(name="const", bufs=1) as cp,
              tc.tile_pool(name="dram", bufs=1, space="DRAM") as dp,
              tc.tile_pool(name="msg", bufs=7) as mp,
              tc.tile_pool(name="oh", bufs=4) as ohp,
              tc.tile_pool(name="stage", bufs=3) as sp):
            eidx1a = cp.tile([P, g0_slots // 16], I16)
            nc.sync.dma_start(out=eidx1a[:], in_=e1a_d.ap())
            dloc1 = cp.tile([P, tot1 // P], BF16)
            nc.sync.dma_start(out=dloc1[:], in_=d1_d.ap())
            iota = cp.tile([P, P], BF16)
            nc.sync.dma_start(out=iota[:], in_=iota_d.ap())
            eidx1b = cp.tile([P, (tot1 - g0_slots) // 16], I16)
            nc.sync.dma_start(out=eidx1b[:], in_=e1b_d.ap())
            xt = cp.tile([P, npad], BF16)
            nc.sync.dma_start(out=xt[:], in_=xtile_d.ap())
            w1n = cp.tile([P, cfg.hid], BF16)
            nc.sync.dma_start(out=w1n[:], in_=w1n_d.ap())
            w1r = cp.tile([P, cfg.hid], BF16)
            nc.sync.dma_start(out=w1r[:], in_=w1r_d.ap())
            b1c = cp.tile([P, 2], F32)
            nc.sync.dma_start(out=b1c[:], in_=b1_d.ap())
            w2n = cp.tile([P, 2 * P], BF16)
            nc.sync.dma_start(out=w2n[:], in_=w2n_d.ap())
            w2r = cp.tile([P, 2 * P], BF16)
            nc.sync.dma_start(out=w2r[:], in_=w2r_d.ap())
            b2r = cp.tile([1, P], BF16)
            nc.sync.dma_start(out=b2r[:], in_=b2_d.ap())
            eidx2 = cp.tile([P, tot2 // 16], I16)
            nc.sync.dma_start(out=eidx2[:], in_=e2_d.ap())
            dloc2 = cp.tile([P, tot2 // P], BF16)
            nc.sync.dma_start(out=dloc2[:], in_=d2_d.ap())
            dinv = cp.tile([P, nt], F32)
            nc.sync.dma_start(out=dinv[:], in_=dinv_d.ap())
            ones1 = cp.tile([1, P], BF16)
            nc.vector.memset(ones1[:], 1.0)
            ident = cp.tile([P, P], BF16)
            make_identity(nc, ident[:])
            hT = cp.tile([P, 2 * npad], BF16)
            plo = cp.tile([P, npad], F32)

            q_locs = [dp.tile([b - a, P], BF16, name=f"qloc{s}")
                      for s, (a, b) in enumerate(cfg.ag_row)]
            q_all = dp.tile([cfg.n_cores * cfg.npad, P], BF16,
                            addr_space="Local")

            def gather_group(layer, g, src_lo, src_hi):
                groups, L, H = (groups1, L1, H1) if layer == 1 else (groups2, L2, H2)
                t0, gg, lo0, hi0 = groups[g]
                if layer == 1:
                    if g == 0:
                        elo = eidx1a[:, lo0 // 16:(lo0 + gg * L) // 16]
                        ehi = eidx1a[:, hi0 // 16:(hi0 + gg * H) // 16] if H else None
                    else:
                        elo = eidx1b[:, (lo0 - g0_slots) // 16:
                                     (lo0 - g0_slots + gg * L) // 16]
                        ehi = (eidx1b[:, (hi0 - g0_slots) // 16:
                                      (hi0 - g0_slots + gg * H) // 16] if H else None)
                else:
                    elo = eidx2[:, lo0 // 16:(lo0 + gg * L) // 16]
                    ehi = eidx2[:, hi0 // 16:(hi0 + gg * H) // 16] if H else None
                mlo = mhi = None
                if src_lo is not None:
                    mlo = mp.tile([P, gg * max(L1, L2)], BF16, tag="mlo")
                    nc.gpsimd.dma_gather(
                        mlo[:, :gg * L].rearrange("p (c e) -> p c e", e=P),
                        src_lo, elo, gg * L, gg * L, P, single_packet=False,
                        queue_num=next_q())
                if H and src_hi is not None:
                    mhi = mp.tile([P, gg * max(H1, H2)], BF16, tag="mhi")
                    nc.gpsimd.dma_gather(
                        mhi[:, :gg * H].rearrange("p (c e) -> p c e", e=P),
                        src_hi, ehi, gg * H, gg * H, P, single_packet=False,
                        queue_num=next_q())
                return mlo, mhi

            def onehot_slab(dloc_t, gc, k):
                """one is_equal builds one-hots for chunks gc..gc+k-1"""
                oh = ohp.tile([P, KOH * P], BF16, tag="oh")
                nc.vector.tensor_tensor(
                    out=oh[:, :k * P].rearrange("p (c e) -> p c e", e=P),
                    in0=iota[:].rearrange("p (o e) -> p o e", o=1)
                        .to_broadcast([P, k, P]),
                    in1=dloc_t[:, gc:gc + k].to_broadcast([P, k, P]),
                    op=OP.is_equal)
                return oh

            def aggregate(layer, g, ti, mlo, mhi, pp, tag, which="both"):
                groups, L, H = (groups1, L1, H1) if layer == 1 else (groups2, L2, H2)
                dloc_t = dloc1 if layer == 1 else dloc2
                t0, gg, lo0, hi0 = groups[g]
                lch, hch = L // P, H // P
                # (msg tile, local chunk, global dloc chunk) runs
                runs = []
                if which in ("both", "lo"):
                    runs.append((mlo, ti * lch, lo0 // P + ti * lch, lch))
                if H and which in ("both", "hi"):
                    runs.append((mhi, ti * hch, hi0 // P + ti * hch, hch))
                ps = pp.tile([P, P], F32, tag=tag, name=f"{tag}{g}_{ti}")
                nch = sum(r[3] for r in runs)
                k = 0
                for (mt, lc0, gc0, cnt) in runs:
                    for j0 in range(0, cnt, KOH):
                        kk = min(KOH, cnt - j0)
                        oh = onehot_slab(dloc_t, gc0 + j0, kk)
                        for j in range(kk):
                            lc = lc0 + j0 + j
                            nc.tensor.matmul(ps[:], lhsT=oh[:, j * P:(j + 1) * P],
                                             rhs=mt[:, lc * P:(lc + 1) * P],
                                             start=(k == 0), stop=(k == nch - 1))
                            k += 1
                return ps

            def layer1_tile(g, ti, mlo, mhi, pp):
                t = groups1[g][0] + ti
                rows = P
                ps_a = aggregate(1, g, ti, mlo, mhi, pp, "psa")
                agg_nm = sp.tile([P, P], BF16, tag="aggnm")
                nc.scalar.activation(agg_nm[:], ps_a[:], AF.Copy,
                                     scale=dinv[:, t:t + 1])
                ps_t = pp.tile([P, P], BF16, tag="pst", name=f"pst{t}")
                nc.tensor.transpose(ps_t[:], agg_nm[:], ident[:])
                aggrT = sp.tile([P, P], BF16, tag="aggrT")
                nc.scalar.activation(aggrT[:], ps_t[:], AF.Copy)
                for h in range(2):
                    ps_h = pp.tile([P, P], F32, tag="psh", name=f"psh{t}_{h}")
                    nc.tensor.matmul(ps_h[:], lhsT=w1n[:, h * P:(h + 1) * P],
                                     rhs=aggrT[:], start=True, stop=False)
                    nc.tensor.matmul(ps_h[:], lhsT=w1r[:, h * P:(h + 1) * P],
                                     rhs=xt[:, t * P:(t + 1) * P],
                                     start=False, stop=True)
                    nc.scalar.activation(hT[:, h * npad + t * P:h * npad + (t + 1) * P],
                                         ps_h[:], AF.Relu, bias=b1c[:, h:h + 1])
                ps_q = pp.tile([P, P], F32, tag="psq", name=f"psq{t}")
                nc.tensor.matmul(ps_q[:], lhsT=hT[:, t * P:(t + 1) * P],
                                 rhs=w2n[:, 0:P], start=True, stop=False)
                nc.tensor.matmul(ps_q[:], lhsT=hT[:, npad + t * P:npad + (t + 1) * P],
                                 rhs=w2n[:, P:2 * P], start=False, stop=True)
                qsb = sp.tile([P, P], BF16, tag="qsb")
                nc.scalar.activation(qsb[:], ps_q[:], AF.Copy)
                s = cfg.split_of_tile(t)
                a, b = cfg.ag_row[s]
                r0 = t * P - a
                nc.sync.dma_start(out=q_locs[s][r0:r0 + rows, :],
                                  in_=qsb[:rows, :])

            def layer2_lo_tile(g, ti, mlo, pp):
                # lo-source aggregate + root term + bias -> plo stash
                t = groups2[g][0] + ti
                ps_g = aggregate(2, g, ti, mlo, None, pp, "psg", which="lo")
                agg_sb = sp.tile([P, P], F32, tag="aggsb")
                nc.scalar.activation(agg_sb[:], ps_g[:], AF.Copy,
                                     scale=dinv[:, t:t + 1])
                ps_r = pp.tile([P, P], F32, tag="psr", name=f"psr{t}")
                nc.tensor.matmul(ps_r[:], lhsT=hT[:, t * P:(t + 1) * P],
                                 rhs=w2r[:, 0:P], start=True, stop=False)
                nc.tensor.matmul(ps_r[:], lhsT=hT[:, npad + t * P:npad + (t + 1) * P],
                                 rhs=w2r[:, P:2 * P], start=False, stop=False)
                nc.tensor.matmul(ps_r[:], lhsT=ones1[:], rhs=b2r[:],
                                 start=False, stop=True)
                nc.vector.tensor_tensor(out=plo[:, t * P:(t + 1) * P],
                                        in0=agg_sb[:], in1=ps_r[:], op=OP.add)

            def layer2_hi_tile(g, ti, mhi, pp):
                t = groups2[g][0] + ti
                rows = P
                ps_g = aggregate(2, g, ti, None, mhi, pp, "psg", which="hi")
                agg_sb = sp.tile([P, P], F32, tag="aggsb")
                nc.scalar.activation(agg_sb[:], ps_g[:], AF.Copy,
                                     scale=dinv[:, t:t + 1])
                osb = sp.tile([P, P], F32, tag="osb")
                nc.vector.tensor_tensor(out=osb[:], in0=agg_sb[:],
                                        in1=plo[:, t * P:(t + 1) * P],
                                        op=OP.add)
                nc.sync.dma_start(out=out_d.ap()[t * P:t * P + rows, :],
                                  in_=osb[:rows, :])

            def trigger_ag(s):
                a, b = cfg.ag_row[s]
                nc.gpsimd.collective_compute(
                    "AllGather", mybir.AluOpType.bypass,
                    replica_groups=[list(range(cfg.n_cores))],
                    ins=[q_locs[s].opt()],
                    outs=[q_all[cfg.n_cores * a:cfg.n_cores * b, :].opt()])

            bounds = np.cumsum(cfg.ag_tiles)
            ag_next = [0]
            with tc.tile_pool(name="ps1", bufs=2, space="PSUM") as pp1:
                for g in range(len(groups1)):
                    mlo, mhi = gather_group(
                        1, g, x_full.ap(),
                        x_full.ap()[cfg.hibase:, :] if H1 else None)
                    for ti in range(groups1[g][1]):
                        layer1_tile(g, ti, mlo, mhi, pp1)
                    t_done = groups1[g][0] + groups1[g][1]
                    while (ag_next[0] < len(cfg.ag_tiles)
                           and t_done >= bounds[ag_next[0]]):
                        trigger_ag(ag_next[0])
                        ag_next[0] += 1
            while ag_next[0] < len(cfg.ag_tiles):
                trigger_ag(ag_next[0])
                ag_next[0] += 1

            with tc.tile_pool(name="ps2", bufs=3, space="PSUM") as pp2:
                for g in range(len(groups2)):
                    mlo, _ = gather_group(2, g, q_all[:cfg.split, :], None)
                    for ti in range(groups2[g][1]):
                        layer2_lo_tile(g, ti, mlo, pp2)
                for g in range(len(groups2)):
                    _, mhi = gather_group(2, g, None, q_all[cfg.hibase:, :])
                    for ti in range(groups2[g][1]):
                        layer2_hi_tile(g, ti, mhi, pp2)

    nc.compile()
    return nc


# --------------------------------------------------------------------------
# entry point
# --------------------------------------------------------------------------

_CACHE = {}


def prepare(inputs, cfg=None):
    x = np.asarray(inputs["x"], np.float32)
    if cfg is None:
        cfg = Cfg(n=x.shape[0])
    budgets, in_maps = pack_inputs(
        cfg, x, inputs["edge_index"],
        inputs["W1_nbr"], inputs["W1_root"], inputs["b1"],
        inputs["W2_nbr"], inputs["W2_root"], inputs["b2"])
    key = (cfg.key(), budgets)
    nc = _CACHE.get(key)
    if nc is None:
        nc = build_program(cfg, budgets)
        _CACHE[key] = nc
    return nc, in_maps, cfg


def kernel(**inputs) -> np.ndarray:
    nc, in_maps, cfg = prepare(inputs)
    res = bass_utils.run_bass_kernel_spmd(
        nc, in_maps, core_ids=list(range(cfg.n_cores)))
    out = np.concatenate([res.results[c]["out"] for c in range(cfg.n_cores)],
                         axis=0)
    return np.ascontiguousarray(out[cfg.newpos], dtype=np.float32)
